# revision 16
# baseline (speedup 1.0000x reference)
"""TRN2 Bass kernel for nn_MaxRollingMeanAttentionProbe (sparse_attention).

Computation (reference):
    y      = relu(x @ w1 + b1)                    # [n, 256]
    logits = y @ queries.T ; vals = y @ values.T  # [n, 16]
    window i of size w: score_i = sum_j softmax(logits[i:i+w])_j * vals[i:i+w]_j
    out    = sum_h max_i score[i, h]              # scalar
Strategy: data-parallel over tokens across 8 NeuronCores with a recomputed
(w-1)-token halo, so no collectives are needed (the softmax shift cancels
exactly within any window).

Per core (one SPMD Tile program):
  pass A: stream host pre-packed fp8-e4m3 x tiles [128 dm, 16 chunks, 512 tok];
          DoubleRow fp8 matmuls (2 k-tiles per instruction, 157 TF/s) accumulate
          yT = relu(w1.T @ xT + b1) in fp32 PSUM. The halo group plus group 0
          are emitted first with their four PSUM chains interleaved at k-pair
          granularity, gated on quartered w1 / split x loads, so the PE starts
          as soon as the first k-chunks land. The combined fp8 [q0,v0,q1,v1,..]
          interleaved probe lhsT yields logits/vals stacked in one [32, g] PSUM
          tile via a single DoubleRow matmul emitted between the next group's
          two MLP chains; the Pool engine evicts it to SBUF and a single
          Scalar-queue DMA regroups both halves into the pass-B layout.
  pass B: layout RLV [128 partitions = 8 subchunks x 16 heads, 2 (l/v), 2048 +
          w - 1 tokens]; processed in chunks as their columns complete (block
          order b0,b1,b3,b2 over the groups keeps all but one chunk off the
          drain path): exp on ScalarE; per-chunk standalone prefix scans with
          the logit scan on DVE and the weighted scan on GpSimd in parallel;
          shifted subtracts split DVE/GpSimd; fast approximate reciprocal and
          a fused multiply+reduce-max (tensor_tensor_reduce) on DVE.
Host: pack/cast inputs (fp8 for matmul operands, pre-scaled to dodge fp8
subnormals), final max/sum + probe descale (tiny).
"""

import numpy as np

# Problem constants (shapes are fixed by the problem spec).
N_TOKENS = 131072
D_MODEL = 2048
D_HID = 256
N_HEADS = 16
N_CORES = 8
P = 128                    # SBUF partitions
G = 512                    # tokens per matmul/DMA group
TPC = N_TOKENS // N_CORES  # window starts per core (16384)
GPC = TPC // G             # groups per core without halo (32)
NSUB = 8                   # subchunks per core in pass B
SUB = TPC // NSUB          # window starts per subchunk (2048)
GPS = SUB // G             # groups per subchunk (4)
ND = D_MODEL // P          # 16 d_model chunks
NH2 = D_HID // P           # 2 hidden halves

SCALE_W = 128.0   # host pre-scale on w1 so fp8 values clear the subnormal range
QV_SCALE = 1024.0  # host pre-scale on [queries; values] for the same reason

_NC_CACHE = {}


def _round_fp32r(a: np.ndarray) -> np.ndarray:
    """Round-to-nearest-even to fp32r (11-bit mantissa), new array."""
    u = np.ascontiguousarray(a, dtype=np.float32).view(np.uint32)
    r = (u + np.uint32(0x800) + ((u >> np.uint32(12)) & np.uint32(1))) & np.uint32(
        0xFFFFF000
    )
    return r.view(np.float32)


def _build(w: int, mmdt: str = "f8dr"):
    import concourse.bacc as bacc
    import concourse.tile as tile
    from concourse import mybir
    from contextlib import ExitStack

    F32 = mybir.dt.float32
    MDT = {"f16": mybir.dt.float16, "bf16": mybir.dt.bfloat16,
           "f32r": mybir.dt.float32r, "f8dr": mybir.dt.float8e4}[mmdt]
    DR = mybir.MatmulPerfMode.DoubleRow if mmdt == "f8dr" else None
    QVS = QV_SCALE if DR is not None else 1.0
    AF = mybir.ActivationFunctionType
    AX = mybir.AxisListType
    ADD = mybir.AluOpType.add
    BYP = mybir.AluOpType.bypass
    MUL = mybir.AluOpType.mult
    MAX = mybir.AluOpType.max
    USE_TTR = False      # bisect: tensor_tensor_reduce suspected in hw hang
    INTERLEAVE_START = True

    NG = -(-(TPC + w - 1) // G)    # groups per core incl. halo
    HAS_HALO = NG > GPC
    SUBLEN = SUB + w - 1           # tokens per subchunk incl. halo
    SPLIT = SUB - w + 1            # starts < SPLIT are always-valid windows
    TW = (SUBLEN + 15) // 16 * 16  # padded pass-B tile width
    LW = min(G, ((w - 1 + 63) // 64) * 64) if HAS_HALO else G
    OFF = 1 if HAS_HALO else 0     # proc index offset of block (0, s=0)
    NP = NG                        # proc steps

    # Block processing order within each subchunk: b2 last so that only the
    # chunk whose columns live in b2 gates on the pass-A drain.
    BLOCKS_ORDER = [b for b in (0, 1, 3, 2) if b < GPS]

    def proc_to_group(p):
        if HAS_HALO and p == 0:
            return NG - 1
        idx = p - OFF
        return GPS * (idx % NSUB) + BLOCKS_ORDER[idx // NSUB]

    # Probe of proc q is emitted during iteration q+1 (fused start: q<=1 -> 2).
    def emit_iter(q):
        return max(q + 1, 2 if (HAS_HALO and NP > 2) else 1)

    nc = bacc.Bacc(
        "TRN2",
        target_bir_lowering=False,
        debug=False,
        enable_asserts=False,
        num_devices=N_CORES,
    )
    xg = nc.dram_tensor("xg", [GPC, P, ND, G], MDT, kind="ExternalInput")
    xh_d = (nc.dram_tensor("xh", [P, ND, LW], MDT, kind="ExternalInput")
            if HAS_HALO else None)
    w1p = nc.dram_tensor("w1p", [P, ND, D_HID], MDT, kind="ExternalInput")
    b1p = nc.dram_tensor("b1p", [P, NH2], F32, kind="ExternalInput")
    # Combined probe weights, column-interleaved [q0, v0, q1, v1, ...] so the
    # probe PSUM partition order matches the fused RLV flush DMA stream.
    if DR is not None:
        qvp = nc.dram_tensor("qvp", [P, NH2, 2 * N_HEADS], MDT, kind="ExternalInput")
    else:
        qvp = nc.dram_tensor(
            "qvp", [P, NH2, 2 * N_HEADS], mybir.dt.float16, kind="ExternalInput"
        )
    res = nc.dram_tensor("res", [P, 2], F32, kind="ExternalOutput")

    with tile.TileContext(nc) as tc, ExitStack() as ctx:
        const = ctx.enter_context(tc.tile_pool(name="const", bufs=1))
        w1_sb = const.tile([P, ND, D_HID], MDT)
        b1_sb = const.tile([P, NH2], F32)
        qv_sb = const.tile([P, NH2, 2 * N_HEADS], qvp.dtype)

        # Persistent pass-B layout: partition s*16+h, dim1 = logits/vals,
        # free dim = token within subchunk s (0..SUBLEN).
        bp = ctx.enter_context(tc.tile_pool(name="bp", bufs=1))
        RLV = bp.tile([P, 2, TW], F32)

        # ---- pass-B chunking ----------------------------------------------
        # Chunk boundaries: aligned so each chunk's gating block is as early
        # as possible; SPLIT separates always-valid starts from the last
        # core's tail starts (classes kept in separate sm columns).
        cb = sorted(
            {0, min(G, SUB), min(max(2 * G + 1 - w, G), SUB),
             min((GPS - 1) * G, SUB),
             min(max(SPLIT, 0), SUB), SUB}
        )
        base_chunks = [(a, e) for a, e in zip(cb, cb[1:]) if e > a]

        def gate_q(a, e):
            """Last proc index whose probe flush this chunk needs."""
            lastcol = e + w - 2
            q = 0
            for b in range(a // G, min(lastcol, SUB - 1) // G + 1):
                pos = BLOCKS_ORDER.index(b)
                q = max(q, OFF + pos * NSUB + NSUB - 1)
            if lastcol >= SUB:
                q = max(q, OFF + NSUB - 1)  # b0 of s+1 fills s's halo cols
                # halo group's own flush is proc 0 -> covered by emit_iter
            return q

        # Split any chunk that gates on the drain into two halves so the two
        # tail pieces pipeline across the engines.
        chunks = []
        for a, e in base_chunks:
            if emit_iter(gate_q(a, e)) >= NP and e - a > 320:
                m = (a + e) // 2
                chunks.append((a, m))
                chunks.append((m, e))
            else:
                chunks.append((a, e))
        NCHK = len(chunks)

        sm = bp.tile([P, 2 * NCHK], F32)

        passb_at = {}  # emission iteration -> [chunk ids]
        for c, (a, e) in enumerate(chunks):
            pe = emit_iter(gate_q(a, e))
            if pe < NP:
                # Stagger chunks that share a readiness point: an oversized
                # vector bundle delays evictions enough to starve the probe
                # PSUM ring and stall the PE.
                while pe < NP - 1 and any(
                    abs(pe - k) < 4 for k in passb_at
                ):
                    pe = min(pe + 4, NP - 1)
            passb_at.setdefault(min(pe, NP), []).append(c)

        xpool = ctx.enter_context(tc.tile_pool(name="xpool", bufs=10))
        ypool = ctx.enter_context(tc.tile_pool(name="ypool", bufs=3))
        stpool = ctx.enter_context(tc.tile_pool(name="stpool", bufs=4))
        pbpool = ctx.enter_context(tc.tile_pool(name="pbpool", bufs=2))
        psy = ctx.enter_context(tc.tile_pool(name="psy", bufs=4, space="PSUM"))
        pslv = ctx.enter_context(tc.tile_pool(name="pslv", bufs=4, space="PSUM"))

        nc.vector.memset(sm[:], -3.0e38)

        # ---------------- pass B chunk (emitted as columns land) -----------
        def emit_passb_chunk(c, tail=False):
            # Engine split: the scan ISA only exists on DVE. During pass A the
            # goal is a small DVE bundle (so probe evictions aren't delayed):
            # GpSimd (slow but idle, ~4 ns/col) takes all element-wise work.
            # In the drain tail the goal is latency: DVE (1.2 ns/col) keeps
            # everything on the critical path, GpSimd prefetches EV/Wn.
            a, e = chunks[c]
            CW = e - a + w - 1               # columns read (incl. w-1 halo)
            ns = e - a                       # window starts in this chunk
            E = pbpool.tile([P, CW], F32, tag="E")
            nc.scalar.activation(E[:], RLV[:, 0, a : a + CW], AF.Exp,
                                 scale=1.0 / QVS)
            EV = pbpool.tile([P, CW], F32, tag="EV")
            ev_eng = nc.vector if tail else nc.gpsimd
            ev_eng.tensor_mul(EV[:], E[:], RLV[:, 1, a : a + CW])
            csZ = pbpool.tile([P, CW + 1], F32, tag="csZ")
            nc.vector.memset(csZ[:, 0:1], 0.0)
            nc.vector.tensor_tensor_scan(
                out=csZ[:, 1 : 1 + CW], data0=E[:], data1=E[:],
                initial=0.0, op0=ADD, op1=BYP,
            )
            csW = pbpool.tile([P, CW + 1], F32, tag="csW")
            ev_eng.memset(csW[:, 0:1], 0.0)
            nc.vector.tensor_tensor_scan(
                out=csW[:, 1 : 1 + CW], data0=EV[:], data1=EV[:],
                initial=0.0, op0=ADD, op1=BYP,
            )
            ez = nc.vector if tail else nc.gpsimd
            Z = pbpool.tile([P, ns], F32, tag="Z")
            ez.tensor_sub(Z[:], csZ[:, w : w + ns], csZ[:, 0:ns])
            Wn = pbpool.tile([P, ns], F32, tag="Wn")
            ev_eng.tensor_sub(Wn[:], csW[:, w : w + ns], csW[:, 0:ns])
            R = pbpool.tile([P, ns], F32, tag="R")
            nc.vector.reciprocal_approx_fast(out=R[:], in_=Z[:])
            S = pbpool.tile([P, ns], F32, tag="S")
            # class-0 (always valid) vs class-1 (invalid on the last core's
            # last subchunk); chunk boundaries never straddle SPLIT.
            col = c if a < SPLIT else NCHK + c
            es = nc.vector if tail else nc.gpsimd
            es.tensor_mul(S[:], Wn[:], R[:])
            # free-dim reduce only exists on DVE
            nc.vector.reduce_max(out=sm[:, col : col + 1], in_=S[:], axis=AX.X)

        # ---------------- probe bundle (for an already-computed group) -----
        def emit_probe(g, yt, gw, last=False):
            # Fixed-size pool tiles (sliced for the halo group) so every pool
            # slot is allocated at its maximum size and PSUM stays bank-aligned.
            lvp = pslv.tile([2 * N_HEADS, G], F32, tag="lvp", name="lvp")[:, 0:gw]
            if DR is not None:
                nc.tensor.matmul(
                    lvp[:], qv_sb[:, :, :], yt[:, :, :],
                    start=True, stop=True, perf_mode=DR,
                )
            else:
                for hh in range(NH2):
                    nc.tensor.matmul(
                        lvp[:], qv_sb[:, hh, :], yt[:, hh, :],
                        start=(hh == 0), stop=(hh == NH2 - 1),
                    )
            st = stpool.tile([2 * N_HEADS, G], F32, tag="st", name="st")[:, 0:gw]
            nc.vector.tensor_copy(out=st[:], in_=lvp[:])  # Pool can't read PSUM
            s, b = g // GPS, g % GPS
            qe = nc.sync if last else nc.scalar
            if s < NSUB:
                col = b * G
                qe.dma_start(
                    out=RLV[s * N_HEADS : (s + 1) * N_HEADS, :, col : col + gw],
                    in_=st[:, :],
                )
            if w > 1 and 0 < s <= NSUB and b == 0:
                h0 = (s - 1) * N_HEADS
                # On the Scalar queue: a waiting DMA issue on Sync would block
                # the x-load prefetch stream behind it for ~a full group.
                qe.dma_start(
                    out=RLV[h0 : h0 + N_HEADS, :, SUB:SUBLEN],
                    in_=st[:, 0 : w - 1],
                )

        # ---------------- MLP helpers --------------------------------------
        def mlp_chain(ypt, xt, hh, gw):
            if DR is not None:
                for dp in range(ND // 2):
                    nc.tensor.matmul(
                        ypt[:],
                        w1_sb[:, 2 * dp : 2 * dp + 2, hh * P : (hh + 1) * P],
                        xt[:, 2 * dp : 2 * dp + 2, 0:gw],
                        start=(dp == 0),
                        stop=(dp == ND // 2 - 1),
                        perf_mode=DR,
                    )
            else:
                for d in range(ND):
                    nc.tensor.matmul(
                        ypt[:],
                        w1_sb[:, d, hh * P : (hh + 1) * P],
                        xt[:, d, 0:gw],
                        start=(d == 0),
                        stop=(d == ND - 1),
                    )

        def relu_evict(yt, ypt, hh):
            nc.scalar.activation(
                yt[:, hh, :], ypt[:], AF.Relu,
                bias=b1_sb[:, hh : hh + 1], scale=1.0 / SCALE_W,
            )

        # ---------------- pass A -------------------------------------------
        pending = []       # [(g, yt, gw)] awaiting probe matmul
        YDT = MDT if DR is not None else qvp.dtype

        # Startup loads: w1 quartered + halo x + group-0 x halves, all on the
        # Sync queue in arrival-priority order; b1/qv from Scalar in parallel.
        g0 = proc_to_group(OFF)
        xt0 = xpool.tile([P, ND, G], MDT, tag="xt")
        if HAS_HALO:
            xth = xpool.tile([P, ND, LW], MDT, tag="xth")
        nq = ND // 4
        nc.sync.dma_start(out=w1_sb[:, 0:nq, :], in_=w1p[:, 0:nq, :])
        if HAS_HALO:
            nc.sync.dma_start(out=xth[:], in_=xh_d[:])
        for q4 in range(1, 4):
            nc.sync.dma_start(
                out=w1_sb[:, q4 * nq : (q4 + 1) * nq, :],
                in_=w1p[:, q4 * nq : (q4 + 1) * nq, :],
            )
        nc.sync.dma_start(out=xt0[:, 0 : ND // 2, :], in_=xg[g0, :, 0 : ND // 2, :])
        nc.sync.dma_start(out=xt0[:, ND // 2 : ND, :], in_=xg[g0, :, ND // 2 :, :])
        nc.scalar.dma_start(out=b1_sb[:], in_=b1p[:])
        nc.scalar.dma_start(out=qv_sb[:], in_=qvp[:])

        # Fused first MLP block: halo + group-0 chains interleaved at k-pair
        # granularity so the PE streams as the quartered loads land.
        if DR is not None:
            chains = []
            if HAS_HALO:
                yth = ypool.tile([P, NH2, G], YDT, tag="yt", name="yth")[:, :, 0:LW]
                pA = psy.tile([P, G], F32, tag="ypsum", name="pA")[:, 0:LW]
                pB = psy.tile([P, G], F32, tag="ypsum", name="pB")[:, 0:LW]
                chains += [(pA, xth, 0, LW), (pB, xth, 1, LW)]
            yt0 = ypool.tile([P, NH2, G], YDT, tag="yt")
            pC = psy.tile([P, G], F32, tag="ypsum")
            pD = psy.tile([P, G], F32, tag="ypsum")
            chains += [(pC, xt0, 0, G), (pD, xt0, 1, G)]
            order = (
                [(dp, ch) for dp in range(ND // 2) for ch in chains]
                if INTERLEAVE_START
                else [(dp, ch) for ch in chains for dp in range(ND // 2)]
            )
            for dp, (pt, xt, hh, gw) in order:
                nc.tensor.matmul(
                    pt[:],
                    w1_sb[:, 2 * dp : 2 * dp + 2, hh * P : (hh + 1) * P],
                    xt[:, 2 * dp : 2 * dp + 2, 0:gw],
                    start=(dp == 0),
                    stop=(dp == ND // 2 - 1),
                    perf_mode=DR,
                )
            if HAS_HALO:
                relu_evict(yth, pA, 0)
                relu_evict(yth, pB, 1)
                pending.append((NG - 1, yth, LW))
            relu_evict(yt0, pC, 0)
            relu_evict(yt0, pD, 1)
            pending.append((g0, yt0, G))
        else:
            # non-fp8 fallback: plain sequential chains
            if HAS_HALO:
                yth = ypool.tile([P, NH2, G], YDT, tag="yt", name="yth")[:, :, 0:LW]
                for hh in range(NH2):
                    pt = psy.tile([P, G], F32, tag="ypsum", name="pth")[:, 0:LW]
                    mlp_chain(pt, xth, hh, LW)
                    relu_evict(yth, pt, hh)
                pending.append((NG - 1, yth, LW))
            yt0 = ypool.tile([P, NH2, G], YDT, tag="yt")
            for hh in range(NH2):
                pt = psy.tile([P, G], F32, tag="ypsum")
                mlp_chain(pt, xt0, hh, G)
                relu_evict(yt0, pt, hh)
            pending.append((g0, yt0, G))

        # Steady-state groups.
        for p in range(OFF + 1, NP):
            g = proc_to_group(p)
            xt = xpool.tile([P, ND, G], MDT, tag="xt")
            nc.sync.dma_start(out=xt[:], in_=xg[g, :, :, :])
            yt = ypool.tile([P, NH2, G], YDT, tag="yt")
            for hh in range(NH2):
                ypt = psy.tile([P, G], F32, tag="ypsum")
                mlp_chain(ypt, xt, hh, G)
                relu_evict(yt, ypt, hh)
                if hh == 0:
                    # Emit pending probes between the two chains: the probe
                    # matmul's inputs are ready, so the PE never stalls, and
                    # the flush leaves ~a group earlier than at end-of-group.
                    for (gp, ytp, gwp) in pending:
                        emit_probe(gp, ytp, gwp)
                    pending = []
            for c in passb_at.get(p, []):
                emit_passb_chunk(c)
            pending.append((g, yt, G))
        # Drain the software pipeline.
        for (gp, ytp, gwp) in pending:
            emit_probe(gp, ytp, gwp, last=True)
        for c in passb_at.get(NP, []):
            emit_passb_chunk(c, tail=True)

        # ---------------- final reduction + store ---------------------------
        res2 = bp.tile([P, 2], F32)
        nc.vector.reduce_max(out=res2[:, 0:1], in_=sm[:, 0:NCHK], axis=AX.X)
        nc.vector.reduce_max(
            out=res2[:, 1:2], in_=sm[:, NCHK : 2 * NCHK], axis=AX.X
        )
        nc.sync.dma_start(out=res[:], in_=res2[:])

    nc.compile()
    return nc


MM_DTYPE = "f8dr"


def _get_nc(w: int):
    key = (w, MM_DTYPE)
    nc = _NC_CACHE.get(key)
    if nc is None:
        nc = _build(w, MM_DTYPE)
        _NC_CACHE[key] = nc
    return nc


def _mm_cast(a: np.ndarray) -> np.ndarray:
    """Convert to the MLP matmul input dtype (host-side rounding)."""
    if MM_DTYPE == "f16":
        return a.astype(np.float16)
    if MM_DTYPE == "f8dr":
        import ml_dtypes

        return a.astype(ml_dtypes.float8_e4m3)
    if MM_DTYPE == "bf16":
        import ml_dtypes

        return a.astype(ml_dtypes.bfloat16)
    return _round_fp32r(a)


def _prep_inputs(x, w1, b1, queries, values, w):
    """Host-side packing: pad + round + transpose into DMA-friendly layouts.
    Returns the per-core in_maps for run_bass_kernel_spmd."""
    NG = -(-(TPC + w - 1) // G)
    NGG = (N_CORES - 1) * GPC + NG  # distinct global groups incl. final halo
    xpad = np.zeros((NGG * G, D_MODEL), dtype=np.float32)
    xpad[:N_TOKENS] = x
    xr = _mm_cast(xpad)
    # [gg, p, d, t] = xpad[gg*G + t, d*128 + p]
    xg_all = np.ascontiguousarray(
        xr.reshape(NGG, G, ND, P).transpose(0, 3, 2, 1)
    )
    w1p = np.ascontiguousarray(
        _mm_cast(w1 * SCALE_W).reshape(ND, P, D_HID).transpose(1, 0, 2)
    )
    b1p = np.ascontiguousarray(np.asarray(b1, np.float32).reshape(NH2, P).T)
    # Combined probe weights, interleaved columns [q0, v0, q1, v1, ...] so
    # the probe PSUM partition stream matches the fused RLV flush layout.
    qv = np.empty((2 * N_HEADS, D_HID), dtype=np.float32)
    qv[0::2] = np.asarray(queries, np.float32)
    qv[1::2] = np.asarray(values, np.float32)
    if MM_DTYPE == "f8dr":
        qvT = _mm_cast(qv * QV_SCALE).T.reshape(NH2, P, 2 * N_HEADS)
    else:
        qvT = qv.astype(np.float16).T.reshape(NH2, P, 2 * N_HEADS)  # [hh, k, m]
    qvp = np.ascontiguousarray(qvT.transpose(1, 0, 2))
    LW = min(G, ((w - 1 + 63) // 64) * 64)
    in_maps = []
    for c in range(N_CORES):
        m = {
            "xg": xg_all[c * GPC : (c + 1) * GPC],
            "w1p": w1p,
            "b1p": b1p,
            "qvp": qvp,
        }
        if NG > GPC:
            m["xh"] = np.ascontiguousarray(
                xg_all[c * GPC + NG - 1][:, :, 0:LW]
            )
        in_maps.append(m)
    return in_maps


def _combine(results, w):
    """Host-side final reduction: per-core [128, 2] -> scalar."""
    qvs = QV_SCALE if MM_DTYPE == "f8dr" else 1.0
    best = np.full(N_HEADS, -np.inf, dtype=np.float64)
    for c in range(N_CORES):
        r = np.asarray(results[c]["res"], dtype=np.float64).reshape(NSUB, N_HEADS, 2)
        if c == N_CORES - 1 and w >= 2:
            r = r.copy()
            r[NSUB - 1, :, 1] = -np.inf  # windows past n - w on the last core
        best = np.maximum(best, r.max(axis=(0, 2)))
    return np.asarray((best / qvs).sum(), dtype=np.float32)


def kernel(x, w1, b1, queries, values, window_size):
    from concourse.bass_utils import run_bass_kernel_spmd

    x = np.asarray(x, dtype=np.float32)
    w1 = np.asarray(w1, dtype=np.float32)
    b1 = np.asarray(b1, dtype=np.float32)
    queries = np.asarray(queries, dtype=np.float32)
    values = np.asarray(values, dtype=np.float32)
    w = int(np.asarray(window_size))
    assert x.shape == (N_TOKENS, D_MODEL), x.shape
    assert 1 <= w <= G + 1  # halo duplication reads at most one group

    key = (w, MM_DTYPE)
    fresh = key not in _NC_CACHE
    nc = _get_nc(w)
    in_maps = _prep_inputs(x, w1, b1, queries, values, w)
    last_err = None
    for attempt in range(4):
        try:
            if fresh:
                # Warm-up run: the first execution after NEFF load has been
                # observed to race input upload; discard it.
                run_bass_kernel_spmd(nc, in_maps, core_ids=list(range(N_CORES)))
                fresh = False
            out = run_bass_kernel_spmd(nc, in_maps, core_ids=list(range(N_CORES)))
            return _combine(out.results, w)
        except Exception as e:  # transient terminal/device failures
            last_err = e
            import time as _time

            # Device-unrecoverable states have been observed to need ~60s.
            _time.sleep(15.0 * (attempt + 1))
    raise last_err


# Optional: expose a traced run for profiling from test harnesses.
def kernel_traced(x, w1, b1, queries, values, window_size, tmpdir=None):
    from concourse.bass_utils import run_bass_kernel_spmd

    w = int(np.asarray(window_size))
    nc = _get_nc(w)
    in_maps = _prep_inputs(
        np.asarray(x, np.float32),
        np.asarray(w1, np.float32),
        np.asarray(b1, np.float32),
        np.asarray(queries, np.float32),
        np.asarray(values, np.float32),
        w,
    )
    out = run_bass_kernel_spmd(
        nc, in_maps, core_ids=list(range(N_CORES)), trace=True, tmpdir=tmpdir
    )
    return _combine(out.results, w), out


# revision 17
# speedup vs baseline: 1.0095x; 1.0095x over previous
"""TRN2 Bass kernel for nn_MaxRollingMeanAttentionProbe (sparse_attention).

Computation (reference):
    y      = relu(x @ w1 + b1)                    # [n, 256]
    logits = y @ queries.T ; vals = y @ values.T  # [n, 16]
    window i of size w: score_i = sum_j softmax(logits[i:i+w])_j * vals[i:i+w]_j
    out    = sum_h max_i score[i, h]              # scalar
Strategy: data-parallel over tokens across 8 NeuronCores with a recomputed
(w-1)-token halo, so no collectives are needed (the softmax shift cancels
exactly within any window).

Per core (one SPMD Tile program):
  pass A: stream host pre-packed fp8-e4m3 x tiles [128 dm, 16 chunks, 512 tok];
          DoubleRow fp8 matmuls (2 k-tiles per instruction, 157 TF/s) accumulate
          yT = relu(w1.T @ xT + b1) in fp32 PSUM. The halo group plus group 0
          are emitted first with their four PSUM chains interleaved at k-pair
          granularity, gated on quartered w1 / split x loads, so the PE starts
          as soon as the first k-chunks land. The combined fp8 [q0,v0,q1,v1,..]
          interleaved probe lhsT yields logits/vals stacked in one [32, g] PSUM
          tile via a single DoubleRow matmul emitted between the next group's
          two MLP chains; the Pool engine evicts it to SBUF and a single
          Scalar-queue DMA regroups both halves into the pass-B layout.
  pass B: layout RLV [128 partitions = 8 subchunks x 16 heads, 2 (l/v), 2048 +
          w - 1 tokens]; processed in chunks as their columns complete (block
          order b0,b1,b3,b2 over the groups keeps all but one chunk off the
          drain path): exp on ScalarE; per-chunk standalone prefix scans with
          the logit scan on DVE and the weighted scan on GpSimd in parallel;
          shifted subtracts split DVE/GpSimd; fast approximate reciprocal and
          a fused multiply+reduce-max (tensor_tensor_reduce) on DVE.
Host: pack/cast inputs (fp8 for matmul operands, pre-scaled to dodge fp8
subnormals), final max/sum + probe descale (tiny).
"""

import numpy as np

# Problem constants (shapes are fixed by the problem spec).
N_TOKENS = 131072
D_MODEL = 2048
D_HID = 256
N_HEADS = 16
N_CORES = 8
P = 128                    # SBUF partitions
G = 512                    # tokens per matmul/DMA group
TPC = N_TOKENS // N_CORES  # window starts per core (16384)
GPC = TPC // G             # groups per core without halo (32)
NSUB = 8                   # subchunks per core in pass B
SUB = TPC // NSUB          # window starts per subchunk (2048)
GPS = SUB // G             # groups per subchunk (4)
ND = D_MODEL // P          # 16 d_model chunks
NH2 = D_HID // P           # 2 hidden halves

SCALE_W = 128.0   # host pre-scale on w1 so fp8 values clear the subnormal range
QV_SCALE = 1024.0  # host pre-scale on [queries; values] for the same reason

_NC_CACHE = {}


def _round_fp32r(a: np.ndarray) -> np.ndarray:
    """Round-to-nearest-even to fp32r (11-bit mantissa), new array."""
    u = np.ascontiguousarray(a, dtype=np.float32).view(np.uint32)
    r = (u + np.uint32(0x800) + ((u >> np.uint32(12)) & np.uint32(1))) & np.uint32(
        0xFFFFF000
    )
    return r.view(np.float32)


def _build(w: int, mmdt: str = "f8dr"):
    import concourse.bacc as bacc
    import concourse.tile as tile
    from concourse import mybir
    from contextlib import ExitStack

    F32 = mybir.dt.float32
    MDT = {"f16": mybir.dt.float16, "bf16": mybir.dt.bfloat16,
           "f32r": mybir.dt.float32r, "f8dr": mybir.dt.float8e4}[mmdt]
    DR = mybir.MatmulPerfMode.DoubleRow if mmdt == "f8dr" else None
    QVS = QV_SCALE if DR is not None else 1.0
    AF = mybir.ActivationFunctionType
    AX = mybir.AxisListType
    ADD = mybir.AluOpType.add
    BYP = mybir.AluOpType.bypass
    MUL = mybir.AluOpType.mult
    MAX = mybir.AluOpType.max
    USE_TTR = False      # bisect: tensor_tensor_reduce suspected in hw hang
    INTERLEAVE_START = True

    NG = -(-(TPC + w - 1) // G)    # groups per core incl. halo
    HAS_HALO = NG > GPC
    SUBLEN = SUB + w - 1           # tokens per subchunk incl. halo
    SPLIT = SUB - w + 1            # starts < SPLIT are always-valid windows
    TW = (SUBLEN + 15) // 16 * 16  # padded pass-B tile width
    LW = min(G, ((w - 1 + 63) // 64) * 64) if HAS_HALO else G
    OFF = 1 if HAS_HALO else 0     # proc index offset of block (0, s=0)
    NP = NG                        # proc steps

    # Block processing order within each subchunk: b2 last so that only the
    # chunk whose columns live in b2 gates on the pass-A drain.
    BLOCKS_ORDER = [b for b in (0, 1, 3, 2) if b < GPS]

    def proc_to_group(p):
        if HAS_HALO and p == 0:
            return NG - 1
        idx = p - OFF
        return GPS * (idx % NSUB) + BLOCKS_ORDER[idx // NSUB]

    # Probe of proc q is emitted during iteration q+1 (fused start: q<=1 -> 2).
    def emit_iter(q):
        return max(q + 1, 2 if (HAS_HALO and NP > 2) else 1)

    nc = bacc.Bacc(
        "TRN2",
        target_bir_lowering=False,
        debug=False,
        enable_asserts=False,
        num_devices=N_CORES,
    )
    xg = nc.dram_tensor("xg", [GPC, P, ND, G], MDT, kind="ExternalInput")
    xh_d = (nc.dram_tensor("xh", [P, ND, LW], MDT, kind="ExternalInput")
            if HAS_HALO else None)
    w1p = nc.dram_tensor("w1p", [P, ND, D_HID], MDT, kind="ExternalInput")
    b1p = nc.dram_tensor("b1p", [P, NH2], F32, kind="ExternalInput")
    # Combined probe weights, column-interleaved [q0, v0, q1, v1, ...] so the
    # probe PSUM partition order matches the fused RLV flush DMA stream.
    if DR is not None:
        qvp = nc.dram_tensor("qvp", [P, NH2, 2 * N_HEADS], MDT, kind="ExternalInput")
    else:
        qvp = nc.dram_tensor(
            "qvp", [P, NH2, 2 * N_HEADS], mybir.dt.float16, kind="ExternalInput"
        )
    res = nc.dram_tensor("res", [P, 2], F32, kind="ExternalOutput")

    with tile.TileContext(nc) as tc, ExitStack() as ctx:
        const = ctx.enter_context(tc.tile_pool(name="const", bufs=1))
        w1_sb = const.tile([P, ND, D_HID], MDT)
        b1_sb = const.tile([P, NH2], F32)
        qv_sb = const.tile([P, NH2, 2 * N_HEADS], qvp.dtype)

        # Persistent pass-B layout: partition s*16+h, dim1 = logits/vals,
        # free dim = token within subchunk s (0..SUBLEN).
        bp = ctx.enter_context(tc.tile_pool(name="bp", bufs=1))
        RLV = bp.tile([P, 2, TW], F32)

        # ---- pass-B chunking ----------------------------------------------
        # Chunk boundaries: aligned so each chunk's gating block is as early
        # as possible; SPLIT separates always-valid starts from the last
        # core's tail starts (classes kept in separate sm columns).
        cb = sorted(
            {0, min(G, SUB), min(max(2 * G + 1 - w, G), SUB),
             min((GPS - 1) * G, SUB),
             min(max(SPLIT, 0), SUB), SUB}
        )
        base_chunks = [(a, e) for a, e in zip(cb, cb[1:]) if e > a]

        def gate_q(a, e):
            """Last proc index whose probe flush this chunk needs."""
            lastcol = e + w - 2
            q = 0
            for b in range(a // G, min(lastcol, SUB - 1) // G + 1):
                pos = BLOCKS_ORDER.index(b)
                q = max(q, OFF + pos * NSUB + NSUB - 1)
            if lastcol >= SUB:
                q = max(q, OFF + NSUB - 1)  # b0 of s+1 fills s's halo cols
                # halo group's own flush is proc 0 -> covered by emit_iter
            return q

        # Split any chunk that gates on the drain into two halves so the two
        # tail pieces pipeline across the engines.
        chunks = []
        for a, e in base_chunks:
            if emit_iter(gate_q(a, e)) >= NP and e - a > 320:
                m = (a + e) // 2
                chunks.append((a, m))
                chunks.append((m, e))
            else:
                chunks.append((a, e))
        NCHK = len(chunks)

        sm = bp.tile([P, 2 * NCHK], F32)

        passb_at = {}  # emission iteration -> [chunk ids]
        for c, (a, e) in enumerate(chunks):
            pe = emit_iter(gate_q(a, e))
            if pe < NP:
                # Stagger chunks that share a readiness point: an oversized
                # vector bundle delays evictions enough to starve the probe
                # PSUM ring and stall the PE.
                while pe < NP - 1 and any(
                    abs(pe - k) < 4 for k in passb_at
                ):
                    pe = min(pe + 4, NP - 1)
            passb_at.setdefault(min(pe, NP), []).append(c)

        xpool = ctx.enter_context(tc.tile_pool(name="xpool", bufs=10))
        ypool = ctx.enter_context(tc.tile_pool(name="ypool", bufs=3))
        stpool = ctx.enter_context(tc.tile_pool(name="stpool", bufs=4))
        pbpool = ctx.enter_context(tc.tile_pool(name="pbpool", bufs=2))
        psy = ctx.enter_context(tc.tile_pool(name="psy", bufs=3, space="PSUM"))
        pslv = ctx.enter_context(tc.tile_pool(name="pslv", bufs=5, space="PSUM"))

        nc.vector.memset(sm[:], -3.0e38)

        # ---------------- pass B chunk (emitted as columns land) -----------
        def passb_stages(c, tail=False):
            # Engine split: the scan ISA only exists on DVE. During pass A the
            # goal is a small DVE bundle (so probe evictions aren't delayed):
            # GpSimd (slow but idle, ~4 ns/col) takes all element-wise work.
            # In the drain tail the goal is latency: DVE (1.2 ns/col) keeps
            # everything on the critical path, GpSimd prefetches EV/Wn.
            a, e = chunks[c]
            CW = e - a + w - 1               # columns read (incl. w-1 halo)
            ns = e - a                       # window starts in this chunk
            # Tail chunks keep everything on the fast DVE (GpSimd work slows
            # concurrent DVE ops ~1.8x via SBUF port contention); pass-A
            # chunks push element-wise work to GpSimd and split their DVE ops
            # across three iterations so probe evictions are never delayed by
            # more than ~a scan.
            ee = nc.vector if tail else nc.gpsimd
            st1 = {}

            def stage1():
                E = pbpool.tile([P, CW], F32, tag="E", name="E")
                nc.scalar.activation(E[:], RLV[:, 0, a : a + CW], AF.Exp,
                                     scale=1.0 / QVS)
                EV = pbpool.tile([P, CW], F32, tag="EV", name="EV")
                ee.tensor_mul(EV[:], E[:], RLV[:, 1, a : a + CW])
                csZ = pbpool.tile([P, CW + 1], F32, tag="csZ", name="csZ")
                nc.vector.memset(csZ[:, 0:1], 0.0)
                nc.vector.tensor_tensor_scan(
                    out=csZ[:, 1 : 1 + CW], data0=E[:], data1=E[:],
                    initial=0.0, op0=ADD, op1=BYP,
                )
                csW = pbpool.tile([P, CW + 1], F32, tag="csW", name="csW")
                ee.memset(csW[:, 0:1], 0.0)
                st1.update(EV=EV, csZ=csZ, csW=csW)

            def stage2():
                EV, csZ, csW = st1["EV"], st1["csZ"], st1["csW"]
                nc.vector.tensor_tensor_scan(
                    out=csW[:, 1 : 1 + CW], data0=EV[:], data1=EV[:],
                    initial=0.0, op0=ADD, op1=BYP,
                )
                Z = pbpool.tile([P, ns], F32, tag="Z", name="Z")
                ee.tensor_sub(Z[:], csZ[:, w : w + ns], csZ[:, 0:ns])
                R = pbpool.tile([P, ns], F32, tag="R", name="R")
                nc.vector.reciprocal_approx_fast(out=R[:], in_=Z[:])
                st1.update(R=R)

            def stage3():
                csW, R = st1["csW"], st1["R"]
                Wn = pbpool.tile([P, ns], F32, tag="Wn", name="Wn")
                ee.tensor_sub(Wn[:], csW[:, w : w + ns], csW[:, 0:ns])
                S = pbpool.tile([P, ns], F32, tag="S", name="S")
                ee.tensor_mul(S[:], Wn[:], R[:])
                # class-0 vs class-1 (invalid on the last core's last
                # subchunk); chunk boundaries never straddle SPLIT.
                col = c if a < SPLIT else NCHK + c
                # free-dim reduce only exists on DVE
                nc.vector.reduce_max(out=sm[:, col : col + 1], in_=S[:],
                                     axis=AX.X)

            return [stage1, stage2, stage3]

        # ---------------- probe bundle (for an already-computed group) -----
        def emit_probe(g, yt, gw, last=False):
            # Fixed-size pool tiles (sliced for the halo group) so every pool
            # slot is allocated at its maximum size and PSUM stays bank-aligned.
            lvp = pslv.tile([2 * N_HEADS, G], F32, tag="lvp", name="lvp")[:, 0:gw]
            if DR is not None:
                nc.tensor.matmul(
                    lvp[:], qv_sb[:, :, :], yt[:, :, :],
                    start=True, stop=True, perf_mode=DR,
                )
            else:
                for hh in range(NH2):
                    nc.tensor.matmul(
                        lvp[:], qv_sb[:, hh, :], yt[:, hh, :],
                        start=(hh == 0), stop=(hh == NH2 - 1),
                    )
            st = stpool.tile([2 * N_HEADS, G], F32, tag="st", name="st")[:, 0:gw]
            nc.vector.tensor_copy(out=st[:], in_=lvp[:])  # Pool can't read PSUM
            s, b = g // GPS, g % GPS
            qe = nc.sync if last else nc.scalar
            if s < NSUB:
                col = b * G
                qe.dma_start(
                    out=RLV[s * N_HEADS : (s + 1) * N_HEADS, :, col : col + gw],
                    in_=st[:, :],
                )
            if w > 1 and 0 < s <= NSUB and b == 0:
                h0 = (s - 1) * N_HEADS
                # On the Scalar queue: a waiting DMA issue on Sync would block
                # the x-load prefetch stream behind it for ~a full group.
                qe.dma_start(
                    out=RLV[h0 : h0 + N_HEADS, :, SUB:SUBLEN],
                    in_=st[:, 0 : w - 1],
                )

        # ---------------- MLP helpers --------------------------------------
        def mlp_chain(ypt, xt, hh, gw):
            if DR is not None:
                for dp in range(ND // 2):
                    nc.tensor.matmul(
                        ypt[:],
                        w1_sb[:, 2 * dp : 2 * dp + 2, hh * P : (hh + 1) * P],
                        xt[:, 2 * dp : 2 * dp + 2, 0:gw],
                        start=(dp == 0),
                        stop=(dp == ND // 2 - 1),
                        perf_mode=DR,
                    )
            else:
                for d in range(ND):
                    nc.tensor.matmul(
                        ypt[:],
                        w1_sb[:, d, hh * P : (hh + 1) * P],
                        xt[:, d, 0:gw],
                        start=(d == 0),
                        stop=(d == ND - 1),
                    )

        def relu_evict(yt, ypt, hh):
            nc.scalar.activation(
                yt[:, hh, :], ypt[:], AF.Relu,
                bias=b1_sb[:, hh : hh + 1], scale=1.0 / SCALE_W,
            )

        # ---------------- pass A -------------------------------------------
        pending = []       # [(g, yt, gw)] awaiting probe matmul
        YDT = MDT if DR is not None else qvp.dtype

        # Startup loads: w1 quartered + halo x + group-0 x halves, all on the
        # Sync queue in arrival-priority order; b1/qv from Scalar in parallel.
        g0 = proc_to_group(OFF)
        xt0 = xpool.tile([P, ND, G], MDT, tag="xt")
        if HAS_HALO:
            xth = xpool.tile([P, ND, LW], MDT, tag="xth")
        nq = ND // 4
        nc.sync.dma_start(out=w1_sb[:, 0:nq, :], in_=w1p[:, 0:nq, :])
        if HAS_HALO:
            nc.sync.dma_start(out=xth[:], in_=xh_d[:])
        for q4 in range(1, 4):
            nc.sync.dma_start(
                out=w1_sb[:, q4 * nq : (q4 + 1) * nq, :],
                in_=w1p[:, q4 * nq : (q4 + 1) * nq, :],
            )
        nc.sync.dma_start(out=xt0[:, 0 : ND // 2, :], in_=xg[g0, :, 0 : ND // 2, :])
        nc.sync.dma_start(out=xt0[:, ND // 2 : ND, :], in_=xg[g0, :, ND // 2 :, :])
        nc.scalar.dma_start(out=b1_sb[:], in_=b1p[:])
        nc.scalar.dma_start(out=qv_sb[:], in_=qvp[:])

        # Fused first MLP block: halo + group-0 chains interleaved at k-pair
        # granularity so the PE streams as the quartered loads land.
        if DR is not None:
            chains = []
            if HAS_HALO:
                yth = ypool.tile([P, NH2, G], YDT, tag="yt", name="yth")[:, :, 0:LW]
                pA = psy.tile([P, G], F32, tag="ypsum", name="pA")[:, 0:LW]
                pB = psy.tile([P, G], F32, tag="ypsum", name="pB")[:, 0:LW]
                chains += [(pA, xth, 0, LW), (pB, xth, 1, LW)]
            yt0 = ypool.tile([P, NH2, G], YDT, tag="yt")
            pC = psy.tile([P, G], F32, tag="ypsum")
            pD = psy.tile([P, G], F32, tag="ypsum")
            chains += [(pC, xt0, 0, G), (pD, xt0, 1, G)]
            order = (
                [(dp, ch) for dp in range(ND // 2) for ch in chains]
                if INTERLEAVE_START
                else [(dp, ch) for ch in chains for dp in range(ND // 2)]
            )
            for dp, (pt, xt, hh, gw) in order:
                nc.tensor.matmul(
                    pt[:],
                    w1_sb[:, 2 * dp : 2 * dp + 2, hh * P : (hh + 1) * P],
                    xt[:, 2 * dp : 2 * dp + 2, 0:gw],
                    start=(dp == 0),
                    stop=(dp == ND // 2 - 1),
                    perf_mode=DR,
                )
            if HAS_HALO:
                relu_evict(yth, pA, 0)
                relu_evict(yth, pB, 1)
                pending.append((NG - 1, yth, LW))
            relu_evict(yt0, pC, 0)
            relu_evict(yt0, pD, 1)
            pending.append((g0, yt0, G))
        else:
            # non-fp8 fallback: plain sequential chains
            if HAS_HALO:
                yth = ypool.tile([P, NH2, G], YDT, tag="yt", name="yth")[:, :, 0:LW]
                for hh in range(NH2):
                    pt = psy.tile([P, G], F32, tag="ypsum", name="pth")[:, 0:LW]
                    mlp_chain(pt, xth, hh, LW)
                    relu_evict(yth, pt, hh)
                pending.append((NG - 1, yth, LW))
            yt0 = ypool.tile([P, NH2, G], YDT, tag="yt")
            for hh in range(NH2):
                pt = psy.tile([P, G], F32, tag="ypsum")
                mlp_chain(pt, xt0, hh, G)
                relu_evict(yt0, pt, hh)
            pending.append((g0, yt0, G))

        # Steady-state groups.
        stage_q = []
        for p in range(OFF + 1, NP):
            g = proc_to_group(p)
            xt = xpool.tile([P, ND, G], MDT, tag="xt")
            nc.sync.dma_start(out=xt[:], in_=xg[g, :, :, :])
            yt = ypool.tile([P, NH2, G], YDT, tag="yt")
            for hh in range(NH2):
                ypt = psy.tile([P, G], F32, tag="ypsum")
                mlp_chain(ypt, xt, hh, G)
                relu_evict(yt, ypt, hh)
                if hh == 0:
                    # Emit pending probes between the two chains: the probe
                    # matmul's inputs are ready, so the PE never stalls, and
                    # the flush leaves ~a group earlier than at end-of-group.
                    for (gp, ytp, gwp) in pending:
                        emit_probe(gp, ytp, gwp)
                    pending = []
            for c in passb_at.get(p, []):
                stage_q.extend(passb_stages(c))
            if stage_q:
                stage_q.pop(0)()
            pending.append((g, yt, G))
        # Drain the software pipeline.
        for (gp, ytp, gwp) in pending:
            emit_probe(gp, ytp, gwp, last=True)
        while stage_q:
            stage_q.pop(0)()
        for c in passb_at.get(NP, []):
            for f in passb_stages(c, tail=True):
                f()

        # ---------------- final reduction + store ---------------------------
        res2 = bp.tile([P, 2], F32)
        nc.vector.reduce_max(out=res2[:, 0:1], in_=sm[:, 0:NCHK], axis=AX.X)
        nc.vector.reduce_max(
            out=res2[:, 1:2], in_=sm[:, NCHK : 2 * NCHK], axis=AX.X
        )
        nc.sync.dma_start(out=res[:], in_=res2[:])

    nc.compile()
    return nc


MM_DTYPE = "f8dr"


def _get_nc(w: int):
    key = (w, MM_DTYPE)
    nc = _NC_CACHE.get(key)
    if nc is None:
        nc = _build(w, MM_DTYPE)
        _NC_CACHE[key] = nc
    return nc


def _mm_cast(a: np.ndarray) -> np.ndarray:
    """Convert to the MLP matmul input dtype (host-side rounding)."""
    if MM_DTYPE == "f16":
        return a.astype(np.float16)
    if MM_DTYPE == "f8dr":
        import ml_dtypes

        return a.astype(ml_dtypes.float8_e4m3)
    if MM_DTYPE == "bf16":
        import ml_dtypes

        return a.astype(ml_dtypes.bfloat16)
    return _round_fp32r(a)


def _prep_inputs(x, w1, b1, queries, values, w):
    """Host-side packing: pad + round + transpose into DMA-friendly layouts.
    Returns the per-core in_maps for run_bass_kernel_spmd."""
    NG = -(-(TPC + w - 1) // G)
    NGG = (N_CORES - 1) * GPC + NG  # distinct global groups incl. final halo
    xpad = np.zeros((NGG * G, D_MODEL), dtype=np.float32)
    xpad[:N_TOKENS] = x
    xr = _mm_cast(xpad)
    # [gg, p, d, t] = xpad[gg*G + t, d*128 + p]
    xg_all = np.ascontiguousarray(
        xr.reshape(NGG, G, ND, P).transpose(0, 3, 2, 1)
    )
    w1p = np.ascontiguousarray(
        _mm_cast(w1 * SCALE_W).reshape(ND, P, D_HID).transpose(1, 0, 2)
    )
    b1p = np.ascontiguousarray(np.asarray(b1, np.float32).reshape(NH2, P).T)
    # Combined probe weights, interleaved columns [q0, v0, q1, v1, ...] so
    # the probe PSUM partition stream matches the fused RLV flush layout.
    qv = np.empty((2 * N_HEADS, D_HID), dtype=np.float32)
    qv[0::2] = np.asarray(queries, np.float32)
    qv[1::2] = np.asarray(values, np.float32)
    if MM_DTYPE == "f8dr":
        qvT = _mm_cast(qv * QV_SCALE).T.reshape(NH2, P, 2 * N_HEADS)
    else:
        qvT = qv.astype(np.float16).T.reshape(NH2, P, 2 * N_HEADS)  # [hh, k, m]
    qvp = np.ascontiguousarray(qvT.transpose(1, 0, 2))
    LW = min(G, ((w - 1 + 63) // 64) * 64)
    in_maps = []
    for c in range(N_CORES):
        m = {
            "xg": xg_all[c * GPC : (c + 1) * GPC],
            "w1p": w1p,
            "b1p": b1p,
            "qvp": qvp,
        }
        if NG > GPC:
            m["xh"] = np.ascontiguousarray(
                xg_all[c * GPC + NG - 1][:, :, 0:LW]
            )
        in_maps.append(m)
    return in_maps


def _combine(results, w):
    """Host-side final reduction: per-core [128, 2] -> scalar."""
    qvs = QV_SCALE if MM_DTYPE == "f8dr" else 1.0
    best = np.full(N_HEADS, -np.inf, dtype=np.float64)
    for c in range(N_CORES):
        r = np.asarray(results[c]["res"], dtype=np.float64).reshape(NSUB, N_HEADS, 2)
        if c == N_CORES - 1 and w >= 2:
            r = r.copy()
            r[NSUB - 1, :, 1] = -np.inf  # windows past n - w on the last core
        best = np.maximum(best, r.max(axis=(0, 2)))
    return np.asarray((best / qvs).sum(), dtype=np.float32)


def kernel(x, w1, b1, queries, values, window_size):
    from concourse.bass_utils import run_bass_kernel_spmd

    x = np.asarray(x, dtype=np.float32)
    w1 = np.asarray(w1, dtype=np.float32)
    b1 = np.asarray(b1, dtype=np.float32)
    queries = np.asarray(queries, dtype=np.float32)
    values = np.asarray(values, dtype=np.float32)
    w = int(np.asarray(window_size))
    assert x.shape == (N_TOKENS, D_MODEL), x.shape
    assert 1 <= w <= G + 1  # halo duplication reads at most one group

    key = (w, MM_DTYPE)
    fresh = key not in _NC_CACHE
    nc = _get_nc(w)
    in_maps = _prep_inputs(x, w1, b1, queries, values, w)
    last_err = None
    for attempt in range(4):
        try:
            if fresh:
                # Warm-up run: the first execution after NEFF load has been
                # observed to race input upload; discard it.
                run_bass_kernel_spmd(nc, in_maps, core_ids=list(range(N_CORES)))
                fresh = False
            out = run_bass_kernel_spmd(nc, in_maps, core_ids=list(range(N_CORES)))
            return _combine(out.results, w)
        except Exception as e:  # transient terminal/device failures
            last_err = e
            import time as _time

            # Device-unrecoverable states have been observed to need ~60s.
            _time.sleep(15.0 * (attempt + 1))
    raise last_err


# Optional: expose a traced run for profiling from test harnesses.
def kernel_traced(x, w1, b1, queries, values, window_size, tmpdir=None):
    from concourse.bass_utils import run_bass_kernel_spmd

    w = int(np.asarray(window_size))
    nc = _get_nc(w)
    in_maps = _prep_inputs(
        np.asarray(x, np.float32),
        np.asarray(w1, np.float32),
        np.asarray(b1, np.float32),
        np.asarray(queries, np.float32),
        np.asarray(values, np.float32),
        w,
    )
    out = run_bass_kernel_spmd(
        nc, in_maps, core_ids=list(range(N_CORES)), trace=True, tmpdir=tmpdir
    )
    return _combine(out.results, w), out
